# revision 12
# baseline (speedup 1.0000x reference)
"""Trainium2 Bass kernel: 2-layer LSTM (B=256, T=512, H=512) -> linear head.

Strategy (v2):
  - Data-parallel over batch: 8 cores x 32 rows each, weights replicated.
  - Per step, per layer: gates computed batch-major with the h-state as the
    *stationary* matmul operand ([128 h-dims x 32 batch] per K-chunk) and the
    transposed weight matrix W^T as the *moving* operand, 4x col-tiled so all
    four 32-partition output groups of one PSUM bank accumulate concurrently.
  - Gate-dim blocks permuted to (i, f, o, g) so one sigmoid covers the bank.
  - x_t / bias folded into the same PSUM group via a K=2 rank-2 matmul
    (lhsT = [x_t; 1] from a preloaded x image, rhs = [w_ih; b] image); the
    first of the four col-tiled K=2 matmuls carries start=True, clearing the
    bank's has_written bits so each step begins fresh (no DVE prime).
  - Epilogue fused on DVE with scalar_tensor_tensor:
      tig = (S_g - 0.5) * S_i          [tanh(x) = 2*sigmoid(2x) - 1; the 2x
      c'  = (tig * 2.0) + f*c           is pre-scaled into the g-gate weights]
    with sigmoid output in bf16 SBUF so DVE ops hit the 2x perf mode.
  - Two-stage software pipeline: PE order per step is
      L1mm(t), tr2(t-1), xb2+W2h(t), tr1(t), W2i(t)
    so layer 1's self-recurrence (the critical cycle) never waits on layer-2
    epilogue artifacts, and PE idle gaps stay below the ~3.4us HAM window
    (no junk warm-keepers needed).
"""

import os
import numpy as np

B, T, H, C = 256, 512, 512, 10
NCORES = 8
BC = B // NCORES  # 32
G4 = 4 * H  # 2048

# gate blocks reordered (i, f, o, g): sigmoid gates contiguous
_PERM = np.concatenate(
    [np.arange(0, 512), np.arange(512, 1024), np.arange(1536, 2048), np.arange(1024, 1536)]
)

_BUILD_CACHE = {}


def _build(t_steps=T):
    """Trace + schedule + compile the bass module. Returns nc."""
    import concourse.bass as bass
    import concourse.tile as tile
    from concourse import bacc, mybir
    from contextlib import ExitStack

    f32 = mybir.dt.float32
    bf16 = mybir.dt.bfloat16
    f16 = mybir.dt.float16
    AF = mybir.ActivationFunctionType
    MULT = mybir.AluOpType.mult
    ADD = mybir.AluOpType.add
    SUB = mybir.AluOpType.subtract
    assert t_steps % 4 == 0
    tq_len = t_steps // 4  # steps handled per q row-pair

    nc = bacc.Bacc("TRN2", target_bir_lowering=False, debug=False, num_devices=NCORES)

    dW1 = nc.dram_tensor("w1t", (128, 4 * G4), bf16, kind="ExternalInput").ap()
    dW2i = nc.dram_tensor("w2it", (128, 4 * G4), bf16, kind="ExternalInput").ap()
    dW2h = nc.dram_tensor("w2ht", (128, 4 * G4), bf16, kind="ExternalInput").ap()
    dXB1 = nc.dram_tensor("xb1", (128, G4), bf16, kind="ExternalInput").ap()
    dXB2 = nc.dram_tensor("xb2", (128, G4), bf16, kind="ExternalInput").ap()
    dXA = nc.dram_tensor("xaug", (128, BC * tq_len), bf16, kind="ExternalInput").ap()
    dWoT = nc.dram_tensor("wot", (128, 4 * C), bf16, kind="ExternalInput").ap()
    dId = nc.dram_tensor("id32", (128, 128), bf16, kind="ExternalInput").ap()
    dY = nc.dram_tensor("y", (BC, C), f32, kind="ExternalOutput").ap()

    def w_ap(tile_ap, k, jh):
        # cols of W^T image: 2048*k + 512*gi + 128*jh + h' ; returns [128, gi=4, h'=128]
        return tile_ap.rearrange("p (k gi j h) -> p k gi j h", k=4, gi=4, j=4, h=128)[
            :, k, :, jh, :
        ]

    def xb_ap(tile_ap, q, jh):
        # [2, gi=4, h'=128] slice of the [128, 2048] (w_ih/bias) image at row pair 32q
        return tile_ap.rearrange("p (gi j h) -> p gi j h", gi=4, j=4, h=128)[
            32 * q : 32 * q + 2, :, jh, :
        ]

    with tile.TileContext(nc) as tc, ExitStack() as ctx:
        const = ctx.enter_context(tc.tile_pool(name="const", bufs=1))
        W1 = const.tile([128, 4 * G4], bf16, tag="w1")
        W2i = const.tile([128, 4 * G4], bf16, tag="w2i")
        W2h = const.tile([128, 4 * G4], bf16, tag="w2h")
        XB1 = const.tile([128, G4], bf16, tag="xb1")
        XB2 = const.tile([128, G4], bf16, tag="xb2")
        XA = const.tile([128, BC * tq_len], bf16, tag="xa")
        WoT = const.tile([128, 4 * C], bf16, tag="wot")
        Id128 = const.tile([128, 128], bf16, tag="id128")
        nc.sync.dma_start(W1[:], dW1)
        nc.sync.dma_start(W2i[:], dW2i)
        nc.sync.dma_start(W2h[:], dW2h)
        nc.sync.dma_start(XB1[:], dXB1)
        nc.sync.dma_start(XB2[:], dXB2)
        nc.sync.dma_start(XA[:], dXA)
        nc.sync.dma_start(WoT[:], dWoT)
        nc.sync.dma_start(Id128[:], dId)

        pg1p = ctx.enter_context(tc.tile_pool(name="pg1", bufs=2, space="PSUM"))
        pg2p = ctx.enter_context(tc.tile_pool(name="pg2", bufs=2, space="PSUM"))
        ptrp = ctx.enter_context(tc.tile_pool(name="ptr", bufs=2, space="PSUM"))
        poutp = ctx.enter_context(tc.tile_pool(name="pout", bufs=1, space="PSUM"))

        sigp = ctx.enter_context(tc.tile_pool(name="sig", bufs=2))
        statep = ctx.enter_context(tc.tile_pool(name="state", bufs=3))
        tmpp = ctx.enter_context(tc.tile_pool(name="tmp", bufs=4))
        outp = ctx.enter_context(tc.tile_pool(name="out", bufs=1))

        h1T = statep.tile([128, 128], f16, tag="h1T")
        h2T = statep.tile([128, 128], f16, tag="h2T")
        c1 = statep.tile([128, 128], f32, tag="c1")
        c2 = statep.tile([128, 128], f32, tag="c2")
        for st in (h1T, h2T, c1, c2):
            nc.vector.memset(st[:], 0.0)

        def gate_matmuls(pg, xa2, xbimg, q, hT, Wimg, stop_at_end=True, start=True):
            # K=2 rank-2 matmuls: x_t * w_row + 1 * bias_row; each clears the
            # has_written bits of its own 32-partition output region
            for jh in range(4):
                nc.tensor.matmul(
                    pg[32 * jh : 32 * jh + 32, :], xa2, xb_ap(xbimg, q, jh),
                    start=start, stop=False,
                    tile_position=(32 * q, 32 * jh),
                    skip_group_check=True,
                )
            for k in range(4):
                for jh in range(4):
                    nc.tensor.matmul(
                        pg[32 * jh : 32 * jh + 32, :],
                        hT[:, 32 * k : 32 * k + 32],
                        w_ap(Wimg, k, jh),
                        start=False,
                        stop=(stop_at_end and k == 3 and jh == 3),
                        tile_position=(0, 32 * jh),
                        skip_group_check=True,
                    )

        def recur_matmuls(pg, hT, Wimg, stop_at_end=True):
            for k in range(4):
                for jh in range(4):
                    nc.tensor.matmul(
                        pg[32 * jh : 32 * jh + 32, :],
                        hT[:, 32 * k : 32 * k + 32],
                        w_ap(Wimg, k, jh),
                        start=False,
                        stop=(stop_at_end and k == 3 and jh == 3),
                        tile_position=(0, 32 * jh),
                        skip_group_check=True,
                    )

        def sigmoid_of(pg, tagsuf):
            S = sigp.tile([128, 512], f16, tag="s" + tagsuf)
            nc.scalar.activation(S[:], pg[:], AF.Sigmoid)
            return S

        def state_math(S, c_prev, tagsuf):
            """fused state update after sigmoid; returns (h_bm bf16, c_new f32)."""
            i_ = S[:, 0:128]
            f_ = S[:, 128:256]
            o_ = S[:, 256:384]
            g_ = S[:, 384:512]
            tig = tmpp.tile([128, 128], f16, tag="tig" + tagsuf)
            nc.vector.scalar_tensor_tensor(tig[:], g_, 0.5, i_, SUB, MULT)
            tfc = tmpp.tile([128, 128], f32, tag="tfc" + tagsuf)
            nc.gpsimd.tensor_mul(tfc[:], f_, c_prev[:])
            c_new = statep.tile([128, 128], f32, tag="c" + tagsuf)
            nc.vector.scalar_tensor_tensor(c_new[:], tig[:], 2.0, tfc[:], MULT, ADD)
            tc_ = tmpp.tile([128, 128], f16, tag="tc" + tagsuf)
            nc.scalar.activation(tc_[:], c_new[:], AF.Tanh)
            hbm = tmpp.tile([128, 128], f16, tag="hbm" + tagsuf)
            nc.vector.tensor_mul(hbm[:], o_, tc_[:])
            return hbm, c_new

        def transpose_cast(hbm, tagsuf):
            pt = ptrp.tile([128, 512], f32, tag="pt" + tagsuf, bufs=1)
            nc.tensor.matmul(pt[:, 0:128], hbm[:], Id128[:], start=True, stop=True,
                             skip_group_check=True)
            hT_new = statep.tile([128, 128], f16, tag="hT" + tagsuf)
            nc.vector.tensor_copy(hT_new[:], pt[:, 0:128])
            return hT_new

        def xa_of(t):
            q, tqi = divmod(t, tq_len)
            return XA[32 * q : 32 * q + 2, BC * tqi : BC * tqi + BC], q

        # Software pipeline: iteration u runs layer-1's epilogue for step u
        # (the critical self-recurrence) and layer-2's epilogue for step u-1,
        # so sigmoid(L1, u+1) is never queued behind L2 work on the ACT FIFO.
        # pg2's accumulation group is ordered [xb2, W2i, W2h] (start-first /
        # stop-last is all that matters) so W2h can consume h2T(u-1), which
        # only materializes mid-iteration.

        # prologue: step-0 layer-1 matmuls, step-0 pg2 bias+input opener
        xa2, q = xa_of(0)
        pg1 = pg1p.tile([128, 512], f32, tag="pg1")
        gate_matmuls(pg1, xa2, XB1, q, h1T, W1)
        pg2 = pg2p.tile([128, 512], f32, tag="pg2")
        for jh in range(4):
            nc.tensor.matmul(
                pg2[32 * jh : 32 * jh + 32, :], xa2, xb_ap(XB2, q, jh),
                start=True, stop=False, tile_position=(32 * q, 32 * jh),
                skip_group_check=True,
            )

        # Phase-gate the scheduler (sim-side only; no runtime waits) so the
        # frozen per-engine order matches the intended steady state — in
        # particular tr1(u)/L1mm(u+1) must precede W2h(u)/xb2 on the PE queue.
        PH = 8

        def ph(u, k):
            return tc.tile_wait_until((u * PH + k) * 0.001)

        junkp = ctx.enter_context(tc.tile_pool(name="junk", bufs=1, space="PSUM"))
        junk = junkp.tile([32, 512], f32, tag="junk")

        def junk_mm(n):
            # HAM warm-keepers: fill PE dependency-wait gaps so the clock
            # stays at 8/8; const operands so they are always ready
            for _ in range(n):
                nc.tensor.matmul(junk[:], Id128[:, 0:32], W1[:, 0:512],
                                 start=True, stop=True, skip_group_check=True)

        pg2_prev = None
        S2p = None
        for u in range(t_steps):
            # 1. sigmoids: sigma1(u) then sigma2(u-1) back-to-back on ACT --
            #    sigma2 fills the ACT idle window while L1's DVE chain runs
            with ph(u, 0):
                S1 = sigmoid_of(pg1, "1")
                if pg2_prev is not None:
                    S2p = sigmoid_of(pg2_prev, "2")
                h1bm, c1 = state_math(S1, c1, "1")
            # 2. transpose to stationary layout
            with ph(u, 1):
                h1T = transpose_cast(h1bm, "1")
                junk_mm(1)
            # 3. next step's layer-1 matmuls (keeps the L1 loop self-paced)
            if u + 1 < t_steps:
                xa2n, qn = xa_of(u + 1)
                with ph(u, 2):
                    pg1 = pg1p.tile([128, 512], f32, tag="pg1")
                    gate_matmuls(pg1, xa2n, XB1, qn, h1T, W1)
            # 4. layer-2 input part for step u
            with ph(u, 3):
                recur_matmuls(pg2, h1T, W2i, stop_at_end=False)
            # 5. layer-2 state update for step u-1
            if pg2_prev is not None:
                with ph(u, 4):
                    h2bm, c2 = state_math(S2p, c2, "2")
                with ph(u, 5):
                    h2T = transpose_cast(h2bm, "2")
                    junk_mm(1)
            # 6. layer-2 recurrent part closes pg2(u)
            with ph(u, 6):
                recur_matmuls(pg2, h2T, W2h, stop_at_end=True)
            pg2_prev = pg2
            # 7. open pg2(u+1) with the bias matmuls (no data deps)
            if u + 1 < t_steps:
                with ph(u, 7):
                    pg2 = pg2p.tile([128, 512], f32, tag="pg2")
                    for jh in range(4):
                        nc.tensor.matmul(
                            pg2[32 * jh : 32 * jh + 32, :], xa2n, xb_ap(XB2, qn, jh),
                            start=True, stop=False, tile_position=(32 * qn, 32 * jh),
                            skip_group_check=True,
                        )
                    junk_mm(1)

        # drain: layer-2 epilogue for the final step
        S2p = sigmoid_of(pg2_prev, "2")
        h2bm, c2 = state_math(S2p, c2, "2")
        h2T = transpose_cast(h2bm, "2")

        # head: y[32,10] = h2 @ W_out.T
        pout = poutp.tile([BC, C], f32, tag="pout")
        for k in range(4):
            nc.tensor.matmul(
                pout[:],
                h2T[:, 32 * k : 32 * k + 32],
                WoT[:, 10 * k : 10 * k + 10],
                start=(k == 0),
                stop=(k == 3),
            )
        ysb = outp.tile([BC, C], f32, tag="ysb")
        nc.vector.tensor_copy(ysb[:], pout[:])
        nc.sync.dma_start(dY, ysb[:])

    nc.compile()
    return nc


def _prep_consts(W_ih1, W_hh1, b_ih1, b_hh1, W_ih2, W_hh2, b_ih2, b_hh2, W_out):
    """Host-side layout transforms shared by all cores."""
    p = _PERM
    w1p = np.asarray(W_hh1, np.float32)[p].copy()  # [2048, 512]
    w2ip = np.asarray(W_ih2, np.float32)[p].copy()
    w2hp = np.asarray(W_hh2, np.float32)[p].copy()

    def wt_img(w):  # -> [128, 4*2048]
        out = np.empty((128, 4 * G4), np.float32)
        for k in range(4):
            out[:, G4 * k : G4 * (k + 1)] = w[:, 128 * k : 128 * (k + 1)].T
        return out

    wih1p = np.asarray(W_ih1, np.float32)[p, 0].copy()
    bias1p = (np.asarray(b_ih1, np.float32) + np.asarray(b_hh1, np.float32))[p].copy()
    bias2p = (np.asarray(b_ih2, np.float32) + np.asarray(b_hh2, np.float32))[p].copy()
    # tanh-as-sigmoid trick: pre-scale the g-gate rows by 2 everywhere
    gsl = slice(1536, 2048)
    w1p[gsl] *= 2.0
    w2ip[gsl] *= 2.0
    w2hp[gsl] *= 2.0
    wih1p[gsl] *= 2.0
    bias1p[gsl] *= 2.0
    bias2p[gsl] *= 2.0

    xb1 = np.zeros((128, G4), np.float32)
    xb2 = np.zeros((128, G4), np.float32)
    for qq in range(4):
        xb1[32 * qq] = wih1p
        xb1[32 * qq + 1] = bias1p
        xb2[32 * qq + 1] = bias2p

    wot = np.empty((128, 4 * C), np.float32)
    wo = np.asarray(W_out, np.float32)
    for k in range(4):
        wot[:, C * k : C * (k + 1)] = wo[:, 128 * k : 128 * (k + 1)].T

    import ml_dtypes

    bf = ml_dtypes.bfloat16
    return {
        "w1t": wt_img(w1p).astype(bf),
        "w2it": wt_img(w2ip).astype(bf),
        "w2ht": wt_img(w2hp).astype(bf),
        "xb1": xb1.astype(bf),
        "xb2": xb2.astype(bf),
        "wot": wot.astype(bf),
        "id32": np.eye(128, dtype=np.float32).astype(bf),
    }


def _prep_xaug(x_shard, t_steps=T):
    """x image [128, 32*(T/4)]: row 32q = x^T flat for its span, row 32q+1 = ones."""
    tq_len = t_steps // 4
    import ml_dtypes

    xa = np.zeros((128, BC * tq_len), ml_dtypes.bfloat16)
    xs = np.asarray(x_shard, np.float32)  # [32, T]
    for qq in range(4):
        span = xs[:, qq * tq_len : (qq + 1) * tq_len]  # [32, tq_len]
        xa[32 * qq] = span.T.reshape(-1).astype(ml_dtypes.bfloat16)  # col 32*tq + b
        xa[32 * qq + 1] = 1.0
    return xa


def _install_ntff_hook():
    """Provide antenv.axon_hooks (absent in this image) so trace=True works."""
    import sys, types
    if "antenv.axon_hooks" in sys.modules:
        return
    try:
        import antenv
        from trn_agent_boot.trn_boot import _ntff_profile_via_ctypes
    except Exception:
        return
    mod = types.ModuleType("antenv.axon_hooks")
    holder = {}
    mod.set_axon_ntff_profile_hook = lambda h: holder.__setitem__("h", h)
    mod.get_axon_ntff_profile_hook = lambda: holder.get("h")
    sys.modules["antenv.axon_hooks"] = mod
    antenv.axon_hooks = mod
    try:
        hook = _ntff_profile_via_ctypes("/opt/axon/libaxon_pjrt.so")
        if hook is not None:
            mod.set_axon_ntff_profile_hook(hook)
    except Exception:
        pass


def kernel(x, W_ih1, W_hh1, b_ih1, b_hh1, W_ih2, W_hh2, b_ih2, b_hh2, W_out, b_out):
    import sys

    for pth in ("/opt/trn_rl_repo", "/root/.axon_site/_ro/trn_rl_repo"):
        if os.path.isdir(pth) and pth not in sys.path:
            sys.path.append(pth)
    from concourse import bass_utils

    if "nc" not in _BUILD_CACHE:
        _BUILD_CACHE["nc"] = _build(T)
    nc = _BUILD_CACHE["nc"]

    consts = _prep_consts(W_ih1, W_hh1, b_ih1, b_hh1, W_ih2, W_hh2, b_ih2, b_hh2, W_out)
    in_maps = []
    for cidx in range(NCORES):
        sl = slice(BC * cidx, BC * (cidx + 1))
        xs = np.asarray(x)[sl]
        in_maps.append({**consts, "xaug": _prep_xaug(xs)})

    trace = bool(int(os.environ.get("KERNEL_TRACE", "0")))
    if trace:
        _install_ntff_hook()
    res = bass_utils.run_bass_kernel_spmd(nc, in_maps, core_ids=list(range(NCORES)), trace=trace)
    _BUILD_CACHE["last_results"] = res

    out = np.empty((B, C), np.float32)
    bo = np.asarray(b_out, np.float32)
    for cidx in range(NCORES):
        out[BC * cidx : BC * (cidx + 1)] = res.results[cidx]["y"] + bo
    return out


# revision 13
# speedup vs baseline: 1.0797x; 1.0797x over previous
"""Trainium2 Bass kernel: 2-layer LSTM (B=256, T=512, H=512) -> linear head.

Strategy (v2):
  - Data-parallel over batch: 8 cores x 32 rows each, weights replicated.
  - Per step, per layer: gates computed batch-major with the h-state as the
    *stationary* matmul operand ([128 h-dims x 32 batch] per K-chunk) and the
    transposed weight matrix W^T as the *moving* operand, 4x col-tiled so all
    four 32-partition output groups of one PSUM bank accumulate concurrently.
  - Gate-dim blocks permuted to (i, f, o, g) so one sigmoid covers the bank.
  - x_t / bias folded into the same PSUM group via a K=2 rank-2 matmul
    (lhsT = [x_t; 1] from a preloaded x image, rhs = [w_ih; b] image); the
    first of the four col-tiled K=2 matmuls carries start=True, clearing the
    bank's has_written bits so each step begins fresh (no DVE prime).
  - Epilogue fused on DVE with scalar_tensor_tensor:
      tig = (S_g - 0.5) * S_i          [tanh(x) = 2*sigmoid(2x) - 1; the 2x
      c'  = (tig * 2.0) + f*c           is pre-scaled into the g-gate weights]
    with sigmoid output in bf16 SBUF so DVE ops hit the 2x perf mode.
  - Two-stage software pipeline: PE order per step is
      L1mm(t), tr2(t-1), xb2+W2h(t), tr1(t), W2i(t)
    so layer 1's self-recurrence (the critical cycle) never waits on layer-2
    epilogue artifacts, and PE idle gaps stay below the ~3.4us HAM window
    (no junk warm-keepers needed).
"""

import os
import numpy as np

B, T, H, C = 256, 512, 512, 10
NCORES = 8
BC = B // NCORES  # 32
G4 = 4 * H  # 2048

# gate blocks reordered (i, f, o, g): sigmoid gates contiguous
_PERM = np.concatenate(
    [np.arange(0, 512), np.arange(512, 1024), np.arange(1536, 2048), np.arange(1024, 1536)]
)

_BUILD_CACHE = {}


def _build(t_steps=T):
    """Trace + schedule + compile the bass module. Returns nc."""
    import concourse.bass as bass
    import concourse.tile as tile
    from concourse import bacc, mybir
    from contextlib import ExitStack

    f32 = mybir.dt.float32
    bf16 = mybir.dt.bfloat16
    f16 = mybir.dt.float16
    AF = mybir.ActivationFunctionType
    MULT = mybir.AluOpType.mult
    ADD = mybir.AluOpType.add
    SUB = mybir.AluOpType.subtract
    assert t_steps % 4 == 0
    tq_len = t_steps // 4  # steps handled per q row-pair

    nc = bacc.Bacc("TRN2", target_bir_lowering=False, debug=False, num_devices=NCORES)

    dW1 = nc.dram_tensor("w1t", (128, 4 * G4), bf16, kind="ExternalInput").ap()
    dW2i = nc.dram_tensor("w2it", (128, 4 * G4), bf16, kind="ExternalInput").ap()
    dW2h = nc.dram_tensor("w2ht", (128, 4 * G4), bf16, kind="ExternalInput").ap()
    dXB1 = nc.dram_tensor("xb1", (128, G4), bf16, kind="ExternalInput").ap()
    dXB2 = nc.dram_tensor("xb2", (128, G4), bf16, kind="ExternalInput").ap()
    dXA = nc.dram_tensor("xaug", (128, BC * tq_len), bf16, kind="ExternalInput").ap()
    dWoT = nc.dram_tensor("wot", (128, 4 * C), bf16, kind="ExternalInput").ap()
    dId = nc.dram_tensor("id32", (128, 128), bf16, kind="ExternalInput").ap()
    dY = nc.dram_tensor("y", (BC, C), f32, kind="ExternalOutput").ap()

    def w_ap(tile_ap, k, jh):
        # cols of W^T image: 2048*k + 512*gi + 128*jh + h' ; returns [128, gi=4, h'=128]
        return tile_ap.rearrange("p (k gi j h) -> p k gi j h", k=4, gi=4, j=4, h=128)[
            :, k, :, jh, :
        ]

    def xb_ap(tile_ap, q, jh):
        # [2, gi=4, h'=128] slice of the [128, 2048] (w_ih/bias) image at row pair 32q
        return tile_ap.rearrange("p (gi j h) -> p gi j h", gi=4, j=4, h=128)[
            32 * q : 32 * q + 2, :, jh, :
        ]

    with tile.TileContext(nc) as tc, ExitStack() as ctx:
        const = ctx.enter_context(tc.tile_pool(name="const", bufs=1))
        W1 = const.tile([128, 4 * G4], bf16, tag="w1")
        W2i = const.tile([128, 4 * G4], bf16, tag="w2i")
        W2h = const.tile([128, 4 * G4], bf16, tag="w2h")
        XB1 = const.tile([128, G4], bf16, tag="xb1")
        XB2 = const.tile([128, G4], bf16, tag="xb2")
        XA = const.tile([128, BC * tq_len], bf16, tag="xa")
        WoT = const.tile([128, 4 * C], bf16, tag="wot")
        Id128 = const.tile([128, 128], bf16, tag="id128")
        nc.sync.dma_start(W1[:], dW1)
        nc.sync.dma_start(W2i[:], dW2i)
        nc.sync.dma_start(W2h[:], dW2h)
        nc.sync.dma_start(XB1[:], dXB1)
        nc.sync.dma_start(XB2[:], dXB2)
        nc.sync.dma_start(XA[:], dXA)
        nc.sync.dma_start(WoT[:], dWoT)
        nc.sync.dma_start(Id128[:], dId)

        pg1p = ctx.enter_context(tc.tile_pool(name="pg1", bufs=2, space="PSUM"))
        pg2p = ctx.enter_context(tc.tile_pool(name="pg2", bufs=2, space="PSUM"))
        ptrp = ctx.enter_context(tc.tile_pool(name="ptr", bufs=2, space="PSUM"))
        poutp = ctx.enter_context(tc.tile_pool(name="pout", bufs=1, space="PSUM"))

        sigp = ctx.enter_context(tc.tile_pool(name="sig", bufs=2))
        statep = ctx.enter_context(tc.tile_pool(name="state", bufs=3))
        tmpp = ctx.enter_context(tc.tile_pool(name="tmp", bufs=4))
        outp = ctx.enter_context(tc.tile_pool(name="out", bufs=1))

        h1T = statep.tile([128, 128], f16, tag="h1T")
        h2T = statep.tile([128, 128], f16, tag="h2T")
        c1 = statep.tile([128, 128], f32, tag="c1")
        c2 = statep.tile([128, 128], f32, tag="c2")
        for st in (h1T, h2T, c1, c2):
            nc.vector.memset(st[:], 0.0)

        def gate_matmuls(pg, xa2, xbimg, q, hT, Wimg, stop_at_end=True, start=True):
            # K=2 rank-2 matmuls: x_t * w_row + 1 * bias_row; each clears the
            # has_written bits of its own 32-partition output region
            for jh in range(4):
                nc.tensor.matmul(
                    pg[32 * jh : 32 * jh + 32, :], xa2, xb_ap(xbimg, q, jh),
                    start=start, stop=False,
                    tile_position=(32 * q, 32 * jh),
                    skip_group_check=True,
                )
            for k in range(4):
                for jh in range(4):
                    nc.tensor.matmul(
                        pg[32 * jh : 32 * jh + 32, :],
                        hT[:, 32 * k : 32 * k + 32],
                        w_ap(Wimg, k, jh),
                        start=False,
                        stop=(stop_at_end and k == 3 and jh == 3),
                        tile_position=(0, 32 * jh),
                        skip_group_check=True,
                    )

        def recur_matmuls(pg, hT, Wimg, stop_at_end=True):
            for k in range(4):
                for jh in range(4):
                    nc.tensor.matmul(
                        pg[32 * jh : 32 * jh + 32, :],
                        hT[:, 32 * k : 32 * k + 32],
                        w_ap(Wimg, k, jh),
                        start=False,
                        stop=(stop_at_end and k == 3 and jh == 3),
                        tile_position=(0, 32 * jh),
                        skip_group_check=True,
                    )

        def sigmoid_of(pg, tagsuf):
            S = sigp.tile([128, 512], f16, tag="s" + tagsuf)
            nc.scalar.activation(S[:], pg[:], AF.Sigmoid)
            return S

        def state_math(S, c_prev, tagsuf):
            """fused state update after sigmoid; returns (h_bm bf16, c_new f32)."""
            i_ = S[:, 0:128]
            f_ = S[:, 128:256]
            o_ = S[:, 256:384]
            g_ = S[:, 384:512]
            tig = tmpp.tile([128, 128], f16, tag="tig" + tagsuf)
            nc.vector.scalar_tensor_tensor(tig[:], g_, 0.5, i_, SUB, MULT)
            tfc = tmpp.tile([128, 128], f32, tag="tfc" + tagsuf)
            nc.vector.tensor_mul(tfc[:], f_, c_prev[:])
            c_new = statep.tile([128, 128], f32, tag="c" + tagsuf)
            nc.vector.scalar_tensor_tensor(c_new[:], tig[:], 2.0, tfc[:], MULT, ADD)
            tc_ = tmpp.tile([128, 128], f16, tag="tc" + tagsuf)
            nc.scalar.activation(tc_[:], c_new[:], AF.Tanh)
            hbm = tmpp.tile([128, 128], f16, tag="hbm" + tagsuf)
            nc.vector.tensor_mul(hbm[:], o_, tc_[:])
            return hbm, c_new

        def transpose_cast(hbm, tagsuf):
            pt = ptrp.tile([128, 512], f32, tag="pt" + tagsuf, bufs=1)
            nc.tensor.matmul(pt[:, 0:128], hbm[:], Id128[:], start=True, stop=True,
                             skip_group_check=True)
            hT_new = statep.tile([128, 128], f16, tag="hT" + tagsuf)
            nc.vector.tensor_copy(hT_new[:], pt[:, 0:128])
            return hT_new

        def xa_of(t):
            q, tqi = divmod(t, tq_len)
            return XA[32 * q : 32 * q + 2, BC * tqi : BC * tqi + BC], q

        # Software pipeline: iteration u runs layer-1's epilogue for step u
        # (the critical self-recurrence) and layer-2's epilogue for step u-1,
        # so sigmoid(L1, u+1) is never queued behind L2 work on the ACT FIFO.
        # pg2's accumulation group is ordered [xb2, W2i, W2h] (start-first /
        # stop-last is all that matters) so W2h can consume h2T(u-1), which
        # only materializes mid-iteration.

        # prologue: step-0 layer-1 matmuls, step-0 pg2 bias+input opener
        xa2, q = xa_of(0)
        pg1 = pg1p.tile([128, 512], f32, tag="pg1")
        gate_matmuls(pg1, xa2, XB1, q, h1T, W1)
        pg2 = pg2p.tile([128, 512], f32, tag="pg2")
        for jh in range(4):
            nc.tensor.matmul(
                pg2[32 * jh : 32 * jh + 32, :], xa2, xb_ap(XB2, q, jh),
                start=True, stop=False, tile_position=(32 * q, 32 * jh),
                skip_group_check=True,
            )

        # Phase-gate the scheduler (sim-side only; no runtime waits) so the
        # frozen per-engine order matches the intended steady state — in
        # particular tr1(u)/L1mm(u+1) must precede W2h(u)/xb2 on the PE queue.
        PH = 8

        def ph(u, k):
            return tc.tile_wait_until((u * PH + k) * 0.001)

        junkp = ctx.enter_context(tc.tile_pool(name="junk", bufs=1, space="PSUM"))
        junk = junkp.tile([32, 512], f32, tag="junk")

        def junk_mm(n):
            # HAM warm-keepers: fill PE dependency-wait gaps so the clock
            # stays at 8/8; const operands so they are always ready
            for _ in range(n):
                nc.tensor.matmul(junk[:], Id128[:, 0:32], W1[:, 0:512],
                                 start=True, stop=True, skip_group_check=True)

        pg2_prev = None
        S2p = None
        for u in range(t_steps):
            # 1. sigmoids: sigma1(u) then sigma2(u-1) back-to-back on ACT --
            #    sigma2 fills the ACT idle window while L1's DVE chain runs
            with ph(u, 0):
                S1 = sigmoid_of(pg1, "1")
                if pg2_prev is not None:
                    S2p = sigmoid_of(pg2_prev, "2")
                h1bm, c1 = state_math(S1, c1, "1")
            # 2. transpose to stationary layout
            with ph(u, 1):
                h1T = transpose_cast(h1bm, "1")
            # 3. next step's layer-1 matmuls (keeps the L1 loop self-paced)
            if u + 1 < t_steps:
                xa2n, qn = xa_of(u + 1)
                with ph(u, 2):
                    pg1 = pg1p.tile([128, 512], f32, tag="pg1")
                    gate_matmuls(pg1, xa2n, XB1, qn, h1T, W1)
            # 4. layer-2 input part for step u
            with ph(u, 3):
                recur_matmuls(pg2, h1T, W2i, stop_at_end=False)
            # 5. layer-2 state update for step u-1
            if pg2_prev is not None:
                with ph(u, 4):
                    h2bm, c2 = state_math(S2p, c2, "2")
                with ph(u, 5):
                    h2T = transpose_cast(h2bm, "2")
            # 6. layer-2 recurrent part closes pg2(u)
            with ph(u, 6):
                recur_matmuls(pg2, h2T, W2h, stop_at_end=True)
            pg2_prev = pg2
            # 7. open pg2(u+1) with the bias matmuls (no data deps)
            if u + 1 < t_steps:
                with ph(u, 7):
                    pg2 = pg2p.tile([128, 512], f32, tag="pg2")
                    for jh in range(4):
                        nc.tensor.matmul(
                            pg2[32 * jh : 32 * jh + 32, :], xa2n, xb_ap(XB2, qn, jh),
                            start=True, stop=False, tile_position=(32 * qn, 32 * jh),
                            skip_group_check=True,
                        )
                    junk_mm(1)

        # drain: layer-2 epilogue for the final step
        S2p = sigmoid_of(pg2_prev, "2")
        h2bm, c2 = state_math(S2p, c2, "2")
        h2T = transpose_cast(h2bm, "2")

        # head: y[32,10] = h2 @ W_out.T
        pout = poutp.tile([BC, C], f32, tag="pout")
        for k in range(4):
            nc.tensor.matmul(
                pout[:],
                h2T[:, 32 * k : 32 * k + 32],
                WoT[:, 10 * k : 10 * k + 10],
                start=(k == 0),
                stop=(k == 3),
            )
        ysb = outp.tile([BC, C], f32, tag="ysb")
        nc.vector.tensor_copy(ysb[:], pout[:])
        nc.sync.dma_start(dY, ysb[:])

    nc.compile()
    return nc


def _prep_consts(W_ih1, W_hh1, b_ih1, b_hh1, W_ih2, W_hh2, b_ih2, b_hh2, W_out):
    """Host-side layout transforms shared by all cores."""
    p = _PERM
    w1p = np.asarray(W_hh1, np.float32)[p].copy()  # [2048, 512]
    w2ip = np.asarray(W_ih2, np.float32)[p].copy()
    w2hp = np.asarray(W_hh2, np.float32)[p].copy()

    def wt_img(w):  # -> [128, 4*2048]
        out = np.empty((128, 4 * G4), np.float32)
        for k in range(4):
            out[:, G4 * k : G4 * (k + 1)] = w[:, 128 * k : 128 * (k + 1)].T
        return out

    wih1p = np.asarray(W_ih1, np.float32)[p, 0].copy()
    bias1p = (np.asarray(b_ih1, np.float32) + np.asarray(b_hh1, np.float32))[p].copy()
    bias2p = (np.asarray(b_ih2, np.float32) + np.asarray(b_hh2, np.float32))[p].copy()
    # tanh-as-sigmoid trick: pre-scale the g-gate rows by 2 everywhere
    gsl = slice(1536, 2048)
    w1p[gsl] *= 2.0
    w2ip[gsl] *= 2.0
    w2hp[gsl] *= 2.0
    wih1p[gsl] *= 2.0
    bias1p[gsl] *= 2.0
    bias2p[gsl] *= 2.0

    xb1 = np.zeros((128, G4), np.float32)
    xb2 = np.zeros((128, G4), np.float32)
    for qq in range(4):
        xb1[32 * qq] = wih1p
        xb1[32 * qq + 1] = bias1p
        xb2[32 * qq + 1] = bias2p

    wot = np.empty((128, 4 * C), np.float32)
    wo = np.asarray(W_out, np.float32)
    for k in range(4):
        wot[:, C * k : C * (k + 1)] = wo[:, 128 * k : 128 * (k + 1)].T

    import ml_dtypes

    bf = ml_dtypes.bfloat16
    return {
        "w1t": wt_img(w1p).astype(bf),
        "w2it": wt_img(w2ip).astype(bf),
        "w2ht": wt_img(w2hp).astype(bf),
        "xb1": xb1.astype(bf),
        "xb2": xb2.astype(bf),
        "wot": wot.astype(bf),
        "id32": np.eye(128, dtype=np.float32).astype(bf),
    }


def _prep_xaug(x_shard, t_steps=T):
    """x image [128, 32*(T/4)]: row 32q = x^T flat for its span, row 32q+1 = ones."""
    tq_len = t_steps // 4
    import ml_dtypes

    xa = np.zeros((128, BC * tq_len), ml_dtypes.bfloat16)
    xs = np.asarray(x_shard, np.float32)  # [32, T]
    for qq in range(4):
        span = xs[:, qq * tq_len : (qq + 1) * tq_len]  # [32, tq_len]
        xa[32 * qq] = span.T.reshape(-1).astype(ml_dtypes.bfloat16)  # col 32*tq + b
        xa[32 * qq + 1] = 1.0
    return xa


def _install_ntff_hook():
    """Provide antenv.axon_hooks (absent in this image) so trace=True works."""
    import sys, types
    if "antenv.axon_hooks" in sys.modules:
        return
    try:
        import antenv
        from trn_agent_boot.trn_boot import _ntff_profile_via_ctypes
    except Exception:
        return
    mod = types.ModuleType("antenv.axon_hooks")
    holder = {}
    mod.set_axon_ntff_profile_hook = lambda h: holder.__setitem__("h", h)
    mod.get_axon_ntff_profile_hook = lambda: holder.get("h")
    sys.modules["antenv.axon_hooks"] = mod
    antenv.axon_hooks = mod
    try:
        hook = _ntff_profile_via_ctypes("/opt/axon/libaxon_pjrt.so")
        if hook is not None:
            mod.set_axon_ntff_profile_hook(hook)
    except Exception:
        pass


def kernel(x, W_ih1, W_hh1, b_ih1, b_hh1, W_ih2, W_hh2, b_ih2, b_hh2, W_out, b_out):
    import sys

    for pth in ("/opt/trn_rl_repo", "/root/.axon_site/_ro/trn_rl_repo"):
        if os.path.isdir(pth) and pth not in sys.path:
            sys.path.append(pth)
    from concourse import bass_utils

    if "nc" not in _BUILD_CACHE:
        _BUILD_CACHE["nc"] = _build(T)
    nc = _BUILD_CACHE["nc"]

    consts = _prep_consts(W_ih1, W_hh1, b_ih1, b_hh1, W_ih2, W_hh2, b_ih2, b_hh2, W_out)
    in_maps = []
    for cidx in range(NCORES):
        sl = slice(BC * cidx, BC * (cidx + 1))
        xs = np.asarray(x)[sl]
        in_maps.append({**consts, "xaug": _prep_xaug(xs)})

    trace = bool(int(os.environ.get("KERNEL_TRACE", "0")))
    if trace:
        _install_ntff_hook()
    res = bass_utils.run_bass_kernel_spmd(nc, in_maps, core_ids=list(range(NCORES)), trace=trace)
    _BUILD_CACHE["last_results"] = res

    out = np.empty((B, C), np.float32)
    bo = np.asarray(b_out, np.float32)
    for cidx in range(NCORES):
        out[BC * cidx : BC * (cidx + 1)] = res.results[cidx]["y"] + bo
    return out


# revision 14
# speedup vs baseline: 1.2051x; 1.1162x over previous
"""Trainium2 Bass kernel: 2-layer LSTM (B=256, T=512, H=512) -> linear head.

Strategy (v2):
  - Data-parallel over batch: 8 cores x 32 rows each, weights replicated.
  - Per step, per layer: gates computed batch-major with the h-state as the
    *stationary* matmul operand ([128 h-dims x 32 batch] per K-chunk) and the
    transposed weight matrix W^T as the *moving* operand, 4x col-tiled so all
    four 32-partition output groups of one PSUM bank accumulate concurrently.
  - Gate-dim blocks permuted to (i, f, o, g) so one sigmoid covers the bank.
  - x_t / bias folded into the same PSUM group via a K=2 rank-2 matmul
    (lhsT = [x_t; 1] from a preloaded x image, rhs = [w_ih; b] image); the
    first of the four col-tiled K=2 matmuls carries start=True, clearing the
    bank's has_written bits so each step begins fresh (no DVE prime).
  - Epilogue fused on DVE with scalar_tensor_tensor:
      tig = (S_g - 0.5) * S_i          [tanh(x) = 2*sigmoid(2x) - 1; the 2x
      c'  = (tig * 2.0) + f*c           is pre-scaled into the g-gate weights]
    with sigmoid output in bf16 SBUF so DVE ops hit the 2x perf mode.
  - Two-stage software pipeline: PE order per step is
      L1mm(t), tr2(t-1), xb2+W2h(t), tr1(t), W2i(t)
    so layer 1's self-recurrence (the critical cycle) never waits on layer-2
    epilogue artifacts, and PE idle gaps stay below the ~3.4us HAM window
    (no junk warm-keepers needed).
"""

import os
import numpy as np

B, T, H, C = 256, 512, 512, 10
NCORES = 8
BC = B // NCORES  # 32
G4 = 4 * H  # 2048

# gate blocks reordered (i, f, o, g): sigmoid gates contiguous
_PERM = np.concatenate(
    [np.arange(0, 512), np.arange(512, 1024), np.arange(1536, 2048), np.arange(1024, 1536)]
)

_BUILD_CACHE = {}


def _build(t_steps=T):
    """Trace + schedule + compile the bass module. Returns nc."""
    import concourse.bass as bass
    import concourse.tile as tile
    from concourse import bacc, mybir
    from contextlib import ExitStack

    f32 = mybir.dt.float32
    bf16 = mybir.dt.bfloat16
    f16 = mybir.dt.float16
    AF = mybir.ActivationFunctionType
    MULT = mybir.AluOpType.mult
    ADD = mybir.AluOpType.add
    SUB = mybir.AluOpType.subtract
    assert t_steps % 4 == 0
    tq_len = t_steps // 4  # steps handled per q row-pair

    nc = bacc.Bacc("TRN2", target_bir_lowering=False, debug=False, num_devices=NCORES)

    dW1 = nc.dram_tensor("w1t", (128, 4 * G4), bf16, kind="ExternalInput").ap()
    dW2i = nc.dram_tensor("w2it", (128, 4 * G4), bf16, kind="ExternalInput").ap()
    dW2h = nc.dram_tensor("w2ht", (128, 4 * G4), bf16, kind="ExternalInput").ap()
    dXB1 = nc.dram_tensor("xb1", (128, G4), bf16, kind="ExternalInput").ap()
    dXB2 = nc.dram_tensor("xb2", (128, G4), bf16, kind="ExternalInput").ap()
    dXA = nc.dram_tensor("xaug", (128, BC * tq_len), bf16, kind="ExternalInput").ap()
    dWoT = nc.dram_tensor("wot", (128, 4 * C), bf16, kind="ExternalInput").ap()
    dId = nc.dram_tensor("id32", (128, 128), bf16, kind="ExternalInput").ap()
    dY = nc.dram_tensor("y", (BC, C), f32, kind="ExternalOutput").ap()

    def w_ap(tile_ap, k, jh):
        # cols of W^T image: 2048*k + 512*gi + 128*jh + h' ; returns [128, gi=4, h'=128]
        return tile_ap.rearrange("p (k gi j h) -> p k gi j h", k=4, gi=4, j=4, h=128)[
            :, k, :, jh, :
        ]

    def xb_ap(tile_ap, q, jh):
        # [2, gi=4, h'=128] slice of the [128, 2048] (w_ih/bias) image at row pair 32q
        return tile_ap.rearrange("p (gi j h) -> p gi j h", gi=4, j=4, h=128)[
            32 * q : 32 * q + 2, :, jh, :
        ]

    with tile.TileContext(nc) as tc, ExitStack() as ctx:
        const = ctx.enter_context(tc.tile_pool(name="const", bufs=1))
        W1 = const.tile([128, 4 * G4], bf16, tag="w1")
        W2i = const.tile([128, 4 * G4], bf16, tag="w2i")
        W2h = const.tile([128, 4 * G4], bf16, tag="w2h")
        XB1 = const.tile([128, G4], bf16, tag="xb1")
        XB2 = const.tile([128, G4], bf16, tag="xb2")
        XA = const.tile([128, BC * tq_len], bf16, tag="xa")
        WoT = const.tile([128, 4 * C], bf16, tag="wot")
        Id128 = const.tile([128, 128], bf16, tag="id128")
        nc.sync.dma_start(W1[:], dW1)
        nc.sync.dma_start(W2i[:], dW2i)
        nc.sync.dma_start(W2h[:], dW2h)
        nc.sync.dma_start(XB1[:], dXB1)
        nc.sync.dma_start(XB2[:], dXB2)
        nc.sync.dma_start(XA[:], dXA)
        nc.sync.dma_start(WoT[:], dWoT)
        nc.sync.dma_start(Id128[:], dId)

        pg1p = ctx.enter_context(tc.tile_pool(name="pg1", bufs=2, space="PSUM"))
        pg2p = ctx.enter_context(tc.tile_pool(name="pg2", bufs=2, space="PSUM"))
        ptrp = ctx.enter_context(tc.tile_pool(name="ptr", bufs=2, space="PSUM"))
        poutp = ctx.enter_context(tc.tile_pool(name="pout", bufs=1, space="PSUM"))

        sigp = ctx.enter_context(tc.tile_pool(name="sig", bufs=2))
        statep = ctx.enter_context(tc.tile_pool(name="state", bufs=3))
        tmpp = ctx.enter_context(tc.tile_pool(name="tmp", bufs=4))
        outp = ctx.enter_context(tc.tile_pool(name="out", bufs=1))

        h1T = statep.tile([128, 128], f16, tag="h1T")
        h2T = statep.tile([128, 128], f16, tag="h2T")
        c1 = statep.tile([128, 128], f32, tag="c1")
        c2 = statep.tile([128, 128], f32, tag="c2")
        for st in (h1T, h2T, c1, c2):
            nc.vector.memset(st[:], 0.0)

        def gate_matmuls(pg, xa2, xbimg, q, hT, Wimg, stop_at_end=True, start=True):
            # K=2 rank-2 matmuls: x_t * w_row + 1 * bias_row; each clears the
            # has_written bits of its own 32-partition output region
            for jh in range(4):
                nc.tensor.matmul(
                    pg[32 * jh : 32 * jh + 32, :], xa2, xb_ap(xbimg, q, jh),
                    start=start, stop=False,
                    tile_position=(32 * q, 32 * jh),
                    skip_group_check=True,
                )
            for k in range(4):
                for jh in range(4):
                    nc.tensor.matmul(
                        pg[32 * jh : 32 * jh + 32, :],
                        hT[:, 32 * k : 32 * k + 32],
                        w_ap(Wimg, k, jh),
                        start=False,
                        stop=(stop_at_end and k == 3 and jh == 3),
                        tile_position=(0, 32 * jh),
                        skip_group_check=True,
                    )

        def recur_matmuls(pg, hT, Wimg, stop_at_end=True):
            for k in range(4):
                for jh in range(4):
                    nc.tensor.matmul(
                        pg[32 * jh : 32 * jh + 32, :],
                        hT[:, 32 * k : 32 * k + 32],
                        w_ap(Wimg, k, jh),
                        start=False,
                        stop=(stop_at_end and k == 3 and jh == 3),
                        tile_position=(0, 32 * jh),
                        skip_group_check=True,
                    )

        def sigmoid_of(pg, tagsuf):
            S = sigp.tile([128, 512], f16, tag="s" + tagsuf)
            nc.scalar.activation(S[:], pg[:], AF.Sigmoid)
            return S

        def state_math(S, c_prev, tagsuf):
            """fused state update after sigmoid; returns (h_bm bf16, c_new f32)."""
            i_ = S[:, 0:128]
            f_ = S[:, 128:256]
            o_ = S[:, 256:384]
            g_ = S[:, 384:512]
            tig = tmpp.tile([128, 128], f16, tag="tig" + tagsuf)
            nc.vector.scalar_tensor_tensor(tig[:], g_, 0.5, i_, SUB, MULT)
            tfc = tmpp.tile([128, 128], f32, tag="tfc" + tagsuf)
            nc.vector.tensor_mul(tfc[:], f_, c_prev[:])
            c_new = statep.tile([128, 128], f32, tag="c" + tagsuf)
            nc.vector.scalar_tensor_tensor(c_new[:], tig[:], 2.0, tfc[:], MULT, ADD)
            tc_ = tmpp.tile([128, 128], f16, tag="tc" + tagsuf)
            nc.scalar.activation(tc_[:], c_new[:], AF.Tanh)
            hbm = tmpp.tile([128, 128], f16, tag="hbm" + tagsuf)
            nc.vector.tensor_mul(hbm[:], o_, tc_[:])
            return hbm, c_new

        def transpose_cast(hbm, tagsuf):
            pt = ptrp.tile([128, 512], f32, tag="pt" + tagsuf, bufs=1)
            nc.tensor.matmul(pt[:, 0:128], hbm[:], Id128[:], start=True, stop=True,
                             skip_group_check=True)
            hT_new = statep.tile([128, 128], f16, tag="hT" + tagsuf)
            nc.vector.tensor_copy(hT_new[:], pt[:, 0:128])
            return hT_new

        def xa_of(t):
            q, tqi = divmod(t, tq_len)
            return XA[32 * q : 32 * q + 2, BC * tqi : BC * tqi + BC], q

        # Software pipeline: iteration u runs layer-1's epilogue for step u
        # (the critical self-recurrence) and layer-2's epilogue for step u-1,
        # so sigmoid(L1, u+1) is never queued behind L2 work on the ACT FIFO.
        # pg2's accumulation group is ordered [xb2, W2i, W2h] (start-first /
        # stop-last is all that matters) so W2h can consume h2T(u-1), which
        # only materializes mid-iteration.

        # prologue: step-0 layer-1 matmuls, step-0 pg2 bias+input opener
        xa2, q = xa_of(0)
        pg1 = pg1p.tile([128, 512], f32, tag="pg1")
        gate_matmuls(pg1, xa2, XB1, q, h1T, W1)
        pg2 = pg2p.tile([128, 512], f32, tag="pg2")
        for jh in range(4):
            nc.tensor.matmul(
                pg2[32 * jh : 32 * jh + 32, :], xa2, xb_ap(XB2, q, jh),
                start=True, stop=False, tile_position=(32 * q, 32 * jh),
                skip_group_check=True,
            )

        # Phase-gate the scheduler (sim-side only; no runtime waits) so the
        # frozen per-engine order matches the intended steady state — in
        # particular tr1(u)/L1mm(u+1) must precede W2h(u)/xb2 on the PE queue.
        PH = 8

        def ph(u, k):
            return tc.tile_wait_until((u * PH + k) * 0.001)

        junkp = ctx.enter_context(tc.tile_pool(name="junk", bufs=1, space="PSUM"))
        junk = junkp.tile([32, 512], f32, tag="junk")

        def junk_mm(n, cols=512):
            # HAM warm-keepers: fill PE dependency-wait gaps so the clock
            # stays at 8/8; const operands so they are always ready. Small
            # cols => fine-grained filling that barely head-blocks real work.
            for _ in range(n):
                nc.tensor.matmul(junk[:, 0:cols], Id128[:, 0:32], W1[:, 0:cols],
                                 start=True, stop=True, skip_group_check=True)

        pg2_prev = None
        S2p = None
        for u in range(t_steps):
            # 1. sigmoids: sigma1(u) then sigma2(u-1) back-to-back on ACT --
            #    sigma2 fills the ACT idle window while L1's DVE chain runs
            with ph(u, 0):
                S1 = sigmoid_of(pg1, "1")
                if pg2_prev is not None:
                    S2p = sigmoid_of(pg2_prev, "2")
                h1bm, c1 = state_math(S1, c1, "1")
            # 2. transpose to stationary layout
            with ph(u, 1):
                h1T = transpose_cast(h1bm, "1")
            # 3. next step's layer-1 matmuls (keeps the L1 loop self-paced)
            if u + 1 < t_steps:
                xa2n, qn = xa_of(u + 1)
                with ph(u, 2):
                    pg1 = pg1p.tile([128, 512], f32, tag="pg1")
                    gate_matmuls(pg1, xa2n, XB1, qn, h1T, W1)
            # 4. layer-2 input part for step u
            with ph(u, 3):
                recur_matmuls(pg2, h1T, W2i, stop_at_end=False)
                junk_mm(12, cols=128)
            # 5. layer-2 state update for step u-1
            if pg2_prev is not None:
                with ph(u, 4):
                    h2bm, c2 = state_math(S2p, c2, "2")
                with ph(u, 5):
                    h2T = transpose_cast(h2bm, "2")
            # 6. layer-2 recurrent part closes pg2(u)
            with ph(u, 6):
                recur_matmuls(pg2, h2T, W2h, stop_at_end=True)
                junk_mm(3, cols=128)
            pg2_prev = pg2
            # 7. open pg2(u+1) with the bias matmuls (no data deps)
            if u + 1 < t_steps:
                with ph(u, 7):
                    pg2 = pg2p.tile([128, 512], f32, tag="pg2")
                    for jh in range(4):
                        nc.tensor.matmul(
                            pg2[32 * jh : 32 * jh + 32, :], xa2n, xb_ap(XB2, qn, jh),
                            start=True, stop=False, tile_position=(32 * qn, 32 * jh),
                            skip_group_check=True,
                        )

        # drain: layer-2 epilogue for the final step
        S2p = sigmoid_of(pg2_prev, "2")
        h2bm, c2 = state_math(S2p, c2, "2")
        h2T = transpose_cast(h2bm, "2")

        # head: y[32,10] = h2 @ W_out.T
        pout = poutp.tile([BC, C], f32, tag="pout")
        for k in range(4):
            nc.tensor.matmul(
                pout[:],
                h2T[:, 32 * k : 32 * k + 32],
                WoT[:, 10 * k : 10 * k + 10],
                start=(k == 0),
                stop=(k == 3),
            )
        ysb = outp.tile([BC, C], f32, tag="ysb")
        nc.vector.tensor_copy(ysb[:], pout[:])
        nc.sync.dma_start(dY, ysb[:])

    nc.compile()
    return nc


def _prep_consts(W_ih1, W_hh1, b_ih1, b_hh1, W_ih2, W_hh2, b_ih2, b_hh2, W_out):
    """Host-side layout transforms shared by all cores."""
    p = _PERM
    w1p = np.asarray(W_hh1, np.float32)[p].copy()  # [2048, 512]
    w2ip = np.asarray(W_ih2, np.float32)[p].copy()
    w2hp = np.asarray(W_hh2, np.float32)[p].copy()

    def wt_img(w):  # -> [128, 4*2048]
        out = np.empty((128, 4 * G4), np.float32)
        for k in range(4):
            out[:, G4 * k : G4 * (k + 1)] = w[:, 128 * k : 128 * (k + 1)].T
        return out

    wih1p = np.asarray(W_ih1, np.float32)[p, 0].copy()
    bias1p = (np.asarray(b_ih1, np.float32) + np.asarray(b_hh1, np.float32))[p].copy()
    bias2p = (np.asarray(b_ih2, np.float32) + np.asarray(b_hh2, np.float32))[p].copy()
    # tanh-as-sigmoid trick: pre-scale the g-gate rows by 2 everywhere
    gsl = slice(1536, 2048)
    w1p[gsl] *= 2.0
    w2ip[gsl] *= 2.0
    w2hp[gsl] *= 2.0
    wih1p[gsl] *= 2.0
    bias1p[gsl] *= 2.0
    bias2p[gsl] *= 2.0

    xb1 = np.zeros((128, G4), np.float32)
    xb2 = np.zeros((128, G4), np.float32)
    for qq in range(4):
        xb1[32 * qq] = wih1p
        xb1[32 * qq + 1] = bias1p
        xb2[32 * qq + 1] = bias2p

    wot = np.empty((128, 4 * C), np.float32)
    wo = np.asarray(W_out, np.float32)
    for k in range(4):
        wot[:, C * k : C * (k + 1)] = wo[:, 128 * k : 128 * (k + 1)].T

    import ml_dtypes

    bf = ml_dtypes.bfloat16
    return {
        "w1t": wt_img(w1p).astype(bf),
        "w2it": wt_img(w2ip).astype(bf),
        "w2ht": wt_img(w2hp).astype(bf),
        "xb1": xb1.astype(bf),
        "xb2": xb2.astype(bf),
        "wot": wot.astype(bf),
        "id32": np.eye(128, dtype=np.float32).astype(bf),
    }


def _prep_xaug(x_shard, t_steps=T):
    """x image [128, 32*(T/4)]: row 32q = x^T flat for its span, row 32q+1 = ones."""
    tq_len = t_steps // 4
    import ml_dtypes

    xa = np.zeros((128, BC * tq_len), ml_dtypes.bfloat16)
    xs = np.asarray(x_shard, np.float32)  # [32, T]
    for qq in range(4):
        span = xs[:, qq * tq_len : (qq + 1) * tq_len]  # [32, tq_len]
        xa[32 * qq] = span.T.reshape(-1).astype(ml_dtypes.bfloat16)  # col 32*tq + b
        xa[32 * qq + 1] = 1.0
    return xa


def _install_ntff_hook():
    """Provide antenv.axon_hooks (absent in this image) so trace=True works."""
    import sys, types
    if "antenv.axon_hooks" in sys.modules:
        return
    try:
        import antenv
        from trn_agent_boot.trn_boot import _ntff_profile_via_ctypes
    except Exception:
        return
    mod = types.ModuleType("antenv.axon_hooks")
    holder = {}
    mod.set_axon_ntff_profile_hook = lambda h: holder.__setitem__("h", h)
    mod.get_axon_ntff_profile_hook = lambda: holder.get("h")
    sys.modules["antenv.axon_hooks"] = mod
    antenv.axon_hooks = mod
    try:
        hook = _ntff_profile_via_ctypes("/opt/axon/libaxon_pjrt.so")
        if hook is not None:
            mod.set_axon_ntff_profile_hook(hook)
    except Exception:
        pass


def kernel(x, W_ih1, W_hh1, b_ih1, b_hh1, W_ih2, W_hh2, b_ih2, b_hh2, W_out, b_out):
    import sys

    for pth in ("/opt/trn_rl_repo", "/root/.axon_site/_ro/trn_rl_repo"):
        if os.path.isdir(pth) and pth not in sys.path:
            sys.path.append(pth)
    from concourse import bass_utils

    if "nc" not in _BUILD_CACHE:
        _BUILD_CACHE["nc"] = _build(T)
    nc = _BUILD_CACHE["nc"]

    consts = _prep_consts(W_ih1, W_hh1, b_ih1, b_hh1, W_ih2, W_hh2, b_ih2, b_hh2, W_out)
    in_maps = []
    for cidx in range(NCORES):
        sl = slice(BC * cidx, BC * (cidx + 1))
        xs = np.asarray(x)[sl]
        in_maps.append({**consts, "xaug": _prep_xaug(xs)})

    trace = bool(int(os.environ.get("KERNEL_TRACE", "0")))
    if trace:
        _install_ntff_hook()
    res = bass_utils.run_bass_kernel_spmd(nc, in_maps, core_ids=list(range(NCORES)), trace=trace)
    _BUILD_CACHE["last_results"] = res

    out = np.empty((B, C), np.float32)
    bo = np.asarray(b_out, np.float32)
    for cidx in range(NCORES):
        out[BC * cidx : BC * (cidx + 1)] = res.results[cidx]["y"] + bo
    return out
